# revision 30
# baseline (speedup 1.0000x reference)
"""Llama attention (B=1, S=2048, H=32, KVH=8, D=128) on 8 Trainium2 NeuronCores.

Strategy: tensor-parallel over heads for QKV/attention; sequence-parallel for
the input gather and the output projection's ReduceScatter. Core c owns q-heads
4c..4c+3 and kv-head c (GQA repeat_interleave => q-head g uses kv-head g//4).

Per-call wire traffic is the dominant cost (axon tunnel ~50-80 MB/s), so only
the activation crosses the wire per call, quantized:

  host:  x -> per-row 12-bit fixed point (3 byte-planes + f32 row scale),
         sharded over cores by seq chunk (~1.5MB/core up)
  chip:  unpack via int16 shifts -> bf16; PE-transpose own chunk -> xT_c;
         AllGather -> full X^T
         Q^T = W-matmuls, K^T, V^T -> V via PE transpose; RoPE in [d, s] layout
         S^T[k,q] = K^T-tile^T @ Q^T   (causal: skip fully-masked k-tiles)
         P^T = exp(scale*S^T - 10)     (global shift; cancels in normalization)
         attn^T[d,q] += lhsT(V[k,d])^T @ P^T[k,q];  l[q] += ones^T @ P^T
         A_c^T = attn^T * 1/l  (kept in SBUF, [512, S] per core)
         O_c[seq, feat] partial = matmul(lhsT=A_c^T-tile, rhs=Wo^T-rows)
         ReduceScatter(add, f32) over cores -> O[seq_c, :] per core
         outQ = per-row int8 quant of O[seq_c, :] + in-band f32 row scales
         (~1MB/core down)
  host:  dequantize + concat seq chunks -> [1, 2048, 4096] fp32

Weights/tables are uploaded to device DRAM once and cached across calls
(fingerprinted); the jitted executable is built once and reused; the previous
call's device output buffer is re-donated to skip a zeros kernel. Inputs whose
mask is not causal fall back to numpy.
"""

import sys

for _p in ("/opt/trn_rl_repo", "/root/.axon_site/_ro/trn_rl_repo"):
    if _p not in sys.path:
        sys.path.insert(0, _p)

import numpy as np
import ml_dtypes

B, S, HID = 1, 2048, 4096
H, KVH, D = 32, 8, 128
THETA = 10000.0
NC = 8                      # cores
HPC = H // NC               # q-heads per core = 4
FC = HPC * D                # features per core = 512
SCH = S // NC               # seq chunk per core = 256
SC = 512                    # seq chunk (matmul N)
NSC = S // SC               # 4
NJ = HID // 128             # 32 contraction tiles
SCALE = 1.0 / np.sqrt(np.float32(D))
EXP_SHIFT = -10.0

_BF16 = ml_dtypes.bfloat16

_compiled = None
_EXEC = None
_STATIC = {"key": None, "arrs": None}
_CAUSAL = {"key": None, "val": None}
_LAST = {"out": None}
_POOL = None


def _build_nc(for_sim=False):
    _s_bufs = 3      # attention score psum banks   (3+2+2+1 = 8 PSUM banks)
    _ps1_bufs = 3    # projection psum banks        (3+2 = 5 in phase 1)
    _p_bufs = 8      # exp(P^T) sbuf tiles in flight
    import concourse.bacc as bacc
    import concourse.mybir as mybir
    import concourse.tile as tile
    from concourse.masks import make_identity

    f32 = mybir.dt.float32
    bf16 = mybir.dt.bfloat16

    nc = bacc.Bacc("TRN2", target_bir_lowering=False, debug=False, num_devices=(1 if for_sim else NC))

    # 12-bit per-row quantized x: plane0 = low bytes of cols 0:2048, plane1 =
    # hi-nibbles (cols 0:2048 in low nibble, cols 2048:4096 in high nibble),
    # plane2 = low bytes' counterpart for cols 2048:4096... see host packer.
    # Last 4 bytes of each row: f32 per-row scale.
    xs = nc.dram_tensor("xs", [SCH, 3 * 2048 + 4], mybir.dt.uint8, kind="ExternalInput")
    wqT = nc.dram_tensor("wqT", [HID, FC], bf16, kind="ExternalInput")
    wkT = nc.dram_tensor("wkT", [HID, D], bf16, kind="ExternalInput")
    wvT = nc.dram_tensor("wvT", [HID, D], bf16, kind="ExternalInput")
    woT = nc.dram_tensor("woT", [FC, HID], bf16, kind="ExternalInput")
    cosT = nc.dram_tensor("cosT", [D, S], bf16, kind="ExternalInput")
    sinT = nc.dram_tensor("sinT", [D, S], bf16, kind="ExternalInput")
    dmask = nc.dram_tensor("dmask", [D, 4 * SC], bf16, kind="ExternalInput")
    # int8 per-row quantized output; last 4 bytes of each row hold the f32 scale
    outQ = nc.dram_tensor("outQ", [SCH, HID + 4], mybir.dt.int8, kind="ExternalOutput")

    agx_in = nc.dram_tensor("agx_in", [HID, SCH], bf16)
    agx_out = nc.dram_tensor("agx_out", [NC * HID, SCH], bf16, addr_space="Shared")
    rs_in = nc.dram_tensor("rs_in", [S, HID], f32)
    rs_out = nc.dram_tensor("rs_out", [SCH, HID], f32)

    Exp = mybir.ActivationFunctionType.Exp

    with tile.TileContext(nc) as tc:
        with (
            tc.tile_pool(name="const", bufs=1) as constp,
            tc.tile_pool(name="wo", bufs=1) as wop,
            tc.tile_pool(name="aoall", bufs=1) as aop_all,
        ):
            cos_sb = constp.tile([D, S], bf16, tag="cos")
            sin_sb = constp.tile([D, S], bf16, tag="sin")
            dm_sb = constp.tile([D, 4 * SC], bf16, tag="dm")
            ident_sb = constp.tile([128, 128], bf16, tag="id")
            ones_sb = constp.tile([128, 1], bf16, tag="ones")
            bias_sb = constp.tile([128, 1], mybir.dt.float32, tag="bias")
            ones_row = constp.tile([1, 128], mybir.dt.float32, tag="ones_row")
            nc.sync.dma_start(out=cos_sb[:], in_=cosT[:])
            nc.sync.dma_start(out=sin_sb[:], in_=sinT[:])
            nc.sync.dma_start(out=dm_sb[:], in_=dmask[:])
            make_identity(nc, ident_sb[:])
            nc.vector.memset(ones_sb[:], 1.0)
            nc.vector.memset(bias_sb[:], EXP_SHIFT)
            nc.vector.memset(ones_row[:], 1.0)

            # A_c^T tiles [128, S] per local q-head, filled by phase 2, read by phase 3
            aoall = [aop_all.tile([128, S], bf16, tag=f"ao{h}", name=f"ao{h}") for h in range(HPC)]

            # ---------------- phase 0: transpose own x chunk, AllGather X^T ----------------
            with (
                tc.tile_pool(name="x0", bufs=1) as x0p,
                tc.tile_pool(name="xtc", bufs=NJ) as xtcp,
                tc.tile_pool(name="ptr0", bufs=2, space="PSUM") as ptr0,
            ):
                i16 = mybir.dt.int16
                HH = HID // 2  # 2048
                xs_sb = []
                for p in range(SCH // 128):
                    rows = slice(p * 128, (p + 1) * 128)
                    tp = x0p.tile([128, 3 * HH], mybir.dt.uint8, tag=f"tp{p}", name=f"tp{p}")
                    nc.sync.dma_start(out=tp[:], in_=xs[rows, 0 : 3 * HH])
                    scx = x0p.tile([128, 1], f32, tag=f"scx{p}", name=f"scx{p}")
                    nc.sync.dma_start(out=scx[:], in_=xs[rows, 3 * HH : 3 * HH + 4].bitcast(f32))
                    bia = x0p.tile([128, 1], f32, tag=f"bia{p}", name=f"bia{p}")
                    nc.vector.tensor_scalar_mul(bia[:], scx[:], -2048.0)
                    t0 = x0p.tile([128, HH], i16, tag=f"t0{p}", name=f"t0{p}")
                    nc.scalar.copy(t0[:], tp[:, 0:HH])
                    t1 = x0p.tile([128, HH], i16, tag=f"t1{p}", name=f"t1{p}")
                    nc.scalar.copy(t1[:], tp[:, HH : 2 * HH])
                    t2 = x0p.tile([128, HH], i16, tag=f"t2{p}", name=f"t2{p}")
                    nc.scalar.copy(t2[:], tp[:, 2 * HH : 3 * HH])
                    # u0 = t0 + (t1 & 0xF) << 8 ; u1 = (t1 >> 4) + (t2 << 4)
                    hi0 = x0p.tile([128, HH], i16, tag=f"hi0{p}", name=f"hi0{p}")
                    nc.vector.tensor_scalar(
                        hi0[:], t1[:], 15, 8,
                        mybir.AluOpType.bitwise_and, mybir.AluOpType.logical_shift_left,
                    )
                    u0 = x0p.tile([128, HH], i16, tag=f"u0{p}", name=f"u0{p}")
                    nc.vector.tensor_add(u0[:], t0[:], hi0[:])
                    lo1 = x0p.tile([128, HH], i16, tag=f"lo1{p}", name=f"lo1{p}")
                    nc.vector.tensor_scalar(
                        lo1[:], t1[:], 4, None, mybir.AluOpType.logical_shift_right,
                    )
                    hi1 = x0p.tile([128, HH], i16, tag=f"hi1{p}", name=f"hi1{p}")
                    nc.vector.tensor_scalar(
                        hi1[:], t2[:], 4, None, mybir.AluOpType.logical_shift_left,
                    )
                    u1 = x0p.tile([128, HH], i16, tag=f"u1{p}", name=f"u1{p}")
                    nc.vector.tensor_add(u1[:], lo1[:], hi1[:])
                    xd = x0p.tile([128, HID], bf16, tag=f"xs{p}", name=f"xs{p}")
                    nc.scalar.activation(
                        xd[:, 0:HH], u0[:],
                        mybir.ActivationFunctionType.Identity,
                        bias=bia[:], scale=scx[:],
                    )
                    nc.scalar.activation(
                        xd[:, HH:HID], u1[:],
                        mybir.ActivationFunctionType.Identity,
                        bias=bia[:], scale=scx[:],
                    )
                    xs_sb.append(xd)
                for j in range(NJ):
                    xtc = xtcp.tile([128, SCH], bf16, tag="xtc", name="xtc")
                    for p in range(SCH // 128):
                        tr = ptr0.tile([128, 128], bf16, tag="tr0", name="tr0")
                        nc.tensor.transpose(tr[:], xs_sb[p][:, j * 128 : (j + 1) * 128], ident_sb[:])
                        nc.scalar.copy(xtc[:, p * 128 : (p + 1) * 128], tr[:])
                    nc.sync.dma_start(out=agx_in[j * 128 : (j + 1) * 128, :], in_=xtc[:])

            if for_sim:
                nc.sync.dma_start(out=agx_out[0:HID, :], in_=agx_in[:])
            else:
                nc.gpsimd.collective_compute(
                    "AllGather",
                    mybir.AluOpType.bypass,
                    replica_groups=[list(range(NC))],
                    ins=[agx_in.ap()],
                    outs=[agx_out.ap()],
                )

            with tc.tile_pool(name="qkv", bufs=1) as qkvp:
                q_sb = [qkvp.tile([D, S], bf16, tag=f"q{h}", name=f"q{h}") for h in range(HPC)]
                k_sb = qkvp.tile([D, S], bf16, tag="k")
                v_sb = qkvp.tile([128, S], bf16, tag="v")  # [seq-part, d] per 128-tile

                # ---------------- phase 1: projections + RoPE ----------------
                with (
                    tc.tile_pool(name="w", bufs=NJ) as wp,
                    tc.tile_pool(name="xt", bufs=NJ) as xtp,
                    tc.tile_pool(name="ps1", bufs=_ps1_bufs, space="PSUM") as ps1,
                    tc.tile_pool(name="pstr", bufs=2, space="PSUM") as pstr,
                    tc.tile_pool(name="rope", bufs=3) as ropep,
                ):
                    wq_sb = [wp.tile([128, FC], bf16, tag="wq", name=f"wq{_}") for _ in range(NJ)]
                    wk_sb = [wp.tile([128, D], bf16, tag="wk", name=f"wk{_}") for _ in range(NJ)]
                    wv_sb = [wp.tile([128, D], bf16, tag="wv", name=f"wv{_}") for _ in range(NJ)]
                    for j in range(NJ):
                        r = slice(j * 128, (j + 1) * 128)
                        nc.sync.dma_start(out=wq_sb[j][:], in_=wqT[r, :])
                        nc.sync.dma_start(out=wk_sb[j][:], in_=wkT[r, :])
                        nc.sync.dma_start(out=wv_sb[j][:], in_=wvT[r, :])

                    def rope(ps, dst_ap, cols):
                        """ps: [128, SC] psum fp32 (feature-major); writes dst_ap (bf16)."""
                        base = ropep.tile([D, SC], bf16, tag="r0", name="r0")
                        nc.scalar.copy(base[:], ps[:])
                        shf = ropep.tile([D, SC], bf16, tag="r1", name="r1")
                        nc.sync.dma_start(out=shf[0:64, :], in_=base[64:128, :])
                        nc.sync.dma_start(out=shf[64:128, :], in_=base[0:64, :])
                        t1 = ropep.tile([D, SC], bf16, tag="r2", name="r2")
                        nc.vector.tensor_mul(t1[:], base[:], cos_sb[:, cols])
                        t2 = ropep.tile([D, SC], bf16, tag="r3", name="r3")
                        nc.vector.tensor_mul(t2[:], shf[:], sin_sb[:, cols])
                        nc.vector.tensor_add(dst_ap, t1[:], t2[:])

                    for sc in range(NSC):  # stream X^T in four 512-seq chunks
                        cols = slice(sc * SC, (sc + 1) * SC)
                        xt_sb = [xtp.tile([128, SC], bf16, tag="xt", name=f"xt{_}") for _ in range(NJ)]
                        for j in range(NJ):
                            for ci in range(SC // SCH):
                                cc = (SC // SCH) * sc + ci
                                nc.sync.dma_start(
                                    out=xt_sb[j][:, ci * SCH : (ci + 1) * SCH],
                                    in_=agx_out[cc * HID + j * 128 : cc * HID + (j + 1) * 128, :],
                                )
                        # Q^T per head
                        for h in range(HPC):
                            ps = ps1.tile([128, SC], f32, tag="ps", name="ps")
                            for j in range(NJ):
                                nc.tensor.matmul(
                                    ps[:],
                                    wq_sb[j][:, h * 128 : (h + 1) * 128],
                                    xt_sb[j][:],
                                    start=(j == 0),
                                    stop=(j == NJ - 1),
                                )
                            rope(ps, q_sb[h][:, cols], cols)
                        # K^T
                        ps = ps1.tile([128, SC], f32, tag="ps", name="ps")
                        for j in range(NJ):
                            nc.tensor.matmul(
                                ps[:], wk_sb[j][:], xt_sb[j][:],
                                start=(j == 0), stop=(j == NJ - 1),
                            )
                        rope(ps, k_sb[:, cols], cols)
                        # V^T then PE-transpose into V
                        ps = ps1.tile([128, SC], f32, tag="ps", name="ps")
                        for j in range(NJ):
                            nc.tensor.matmul(
                                ps[:], wv_sb[j][:], xt_sb[j][:],
                                start=(j == 0), stop=(j == NJ - 1),
                            )
                        vt = ropep.tile([D, SC], bf16, tag="vt", name="vt")
                        nc.scalar.copy(vt[:], ps[:])
                        for t in range(SC // 128):
                            st = sc * (SC // 128) + t
                            trp = pstr.tile([128, 128], bf16, tag="tr", name="tr")
                            nc.tensor.transpose(
                                trp[:], vt[:, t * 128 : (t + 1) * 128], ident_sb[:]
                            )
                            nc.scalar.copy(v_sb[:, st * 128 : (st + 1) * 128], trp[:])

                # prefetch Wo^T row-slice (overlaps attention): 4 tiles [128, HID]
                wo_sb = [wop.tile([128, HID], bf16, tag=f"wo{_}", name=f"wo{_}") for _ in range(FC // 128)]
                for j in range(FC // 128):
                    nc.sync.dma_start(out=wo_sb[j][:], in_=woT[j * 128 : (j + 1) * 128, :])

                # ---------------- phase 2: causal attention ----------------
                with (
                    tc.tile_pool(name="s", bufs=_s_bufs, space="PSUM") as sp,
                    tc.tile_pool(name="att", bufs=2, space="PSUM") as attp,
                    tc.tile_pool(name="l", bufs=2, space="PSUM") as lp,
                    tc.tile_pool(name="p", bufs=_p_bufs) as pp,
                    tc.tile_pool(name="rc", bufs=3) as rcp,
                    tc.tile_pool(name="bc", bufs=1, space="PSUM") as bcp,
                    tc.tile_pool(name="bcs", bufs=2) as bcsp,
                ):
                    for qc in range(NSC):
                        qcols = slice(qc * SC, (qc + 1) * SC)
                        nkt = 4 * (qc + 1)
                        for g in range(HPC // 2):
                            pair = (2 * g, 2 * g + 1)
                            att_ps = {h: attp.tile([D, SC], f32, tag="att", name=f"att{h}") for h in pair}
                            l_ps = {h: lp.tile([1, SC], f32, tag="l", name=f"l{h}") for h in pair}
                            for kt in range(nkt):
                                kcols = slice(kt * 128, (kt + 1) * 128)
                                s_ps, p_sb = {}, {}
                                for h in pair:
                                    s_ps[h] = sp.tile([128, SC], f32, tag="s", name=f"s{h}")
                                    nc.tensor.matmul(
                                        s_ps[h][:], k_sb[:, kcols], q_sb[h][:, qcols],
                                        start=True, stop=True,
                                    )
                                for h in pair:
                                    p_sb[h] = pp.tile([128, SC], bf16, tag="p", name=f"p{h}")
                                    nc.scalar.activation(
                                        p_sb[h][:], s_ps[h][:], Exp,
                                        bias=bias_sb[:], scale=float(SCALE),
                                    )
                                    jd = kt - 4 * qc
                                    if jd >= 0:
                                        nc.vector.tensor_mul(
                                            p_sb[h][:], p_sb[h][:],
                                            dm_sb[:, jd * SC : (jd + 1) * SC],
                                        )
                                first, last = kt == 0, kt == nkt - 1
                                for h in pair:
                                    nc.tensor.matmul(
                                        att_ps[h][:], v_sb[:, kcols], p_sb[h][:],
                                        start=first, stop=last,
                                    )
                                    nc.tensor.matmul(
                                        l_ps[h][:], ones_sb[:, 0:1], p_sb[h][:],
                                        start=first, stop=last,
                                    )
                            for h in pair:
                                rc = rcp.tile([1, SC], f32, tag="rc", name="rc")
                                nc.vector.reciprocal(rc[:], l_ps[h][:])
                                bc = bcp.tile([D, SC], f32, tag="bc", name="bc")
                                nc.tensor.matmul(bc[:], ones_row[:], rc[:], start=True, stop=True)
                                bcs = bcsp.tile([D, SC], bf16, tag="bcs", name="bcs")
                                nc.scalar.copy(bcs[:], bc[:])
                                nc.vector.tensor_mul(aoall[h][:, qcols], att_ps[h][:], bcs[:])

            # ---------------- phase 3: partial output projection, seq-major ----------------
            with (
                tc.tile_pool(name="ps3", bufs=6, space="PSUM") as ps3,
                tc.tile_pool(name="strip", bufs=3) as stripp,
            ):
                for st in range(S // 128):  # 16 seq row-tiles
                    strip = stripp.tile([128, HID], f32, tag="strip", name="strip")
                    for fc in range(HID // SC):  # 8 output-feature chunks of 512
                        ps = ps3.tile([128, SC], f32, tag="ps3", name="ps3")
                        for j in range(FC // 128):  # contract over this core's 512 A-features
                            nc.tensor.matmul(
                                ps[:],
                                aoall[j][:, st * 128 : (st + 1) * 128],
                                wo_sb[j][:, fc * SC : (fc + 1) * SC],
                                start=(j == 0),
                                stop=(j == FC // 128 - 1),
                            )
                        nc.scalar.copy(strip[:, fc * SC : (fc + 1) * SC], ps[:])
                    nc.sync.dma_start(out=rs_in[st * 128 : (st + 1) * 128, :], in_=strip[:])

            if for_sim:
                nc.sync.dma_start(out=rs_out[:], in_=rs_in[0:SCH, :])
            else:
                nc.gpsimd.collective_compute(
                    "ReduceScatter",
                    mybir.AluOpType.add,
                    replica_groups=[list(range(NC))],
                    ins=[rs_in.ap()],
                    outs=[rs_out.ap()],
                )

            # -------- phase 4: per-row int8 quantization of own seq chunk --------
            QMAX = 126.5
            with (
                tc.tile_pool(name="fin", bufs=2) as finp,
                tc.tile_pool(name="fino", bufs=2) as finop,
                tc.tile_pool(name="fsc", bufs=2) as fscp,
            ):
                for p in range(SCH // 128):
                    rows = slice(p * 128, (p + 1) * 128)
                    rf = finp.tile([128, HID], f32, tag="rf", name="rf")
                    nc.sync.dma_start(out=rf[:], in_=rs_out[rows, :])
                    am = fscp.tile([128, 1], f32, tag="am", name="am")
                    nc.vector.tensor_reduce(
                        am[:], rf[:], mybir.AxisListType.X, mybir.AluOpType.max,
                        apply_absolute_value=True,
                    )
                    am2 = fscp.tile([128, 1], f32, tag="am2", name="am2")
                    nc.vector.tensor_scalar_max(am2[:], am[:], 1e-30)
                    inv = fscp.tile([128, 1], f32, tag="inv", name="inv")
                    nc.vector.reciprocal(inv[:], am2[:])
                    invq = fscp.tile([128, 1], f32, tag="invq", name="invq")
                    nc.vector.tensor_scalar_mul(invq[:], inv[:], float(QMAX))
                    scq = fscp.tile([128, 1], f32, tag="scq", name="scq")
                    nc.vector.tensor_scalar_mul(scq[:], am2[:], float(1.0 / QMAX))
                    ob = finop.tile([128, HID], mybir.dt.int8, tag="ob", name="ob")
                    nc.scalar.activation(
                        ob[:], rf[:], mybir.ActivationFunctionType.Identity,
                        bias=0.0, scale=invq[:],
                    )
                    nc.sync.dma_start(out=outQ[rows, 0:HID], in_=ob[:])
                    nc.sync.dma_start(
                        out=outQ[rows, HID : HID + 4].bitcast(f32), in_=scq[:]
                    )

    nc.compile()
    return nc


def _get_compiled():
    global _compiled
    if _compiled is None:
        _compiled = _build_nc()
    return _compiled


def _get_exec():
    global _EXEC
    if _EXEC is not None:
        return _EXEC
    import jax
    import jax.numpy as jnp
    from jax.sharding import Mesh, PartitionSpec, NamedSharding
    from jax.experimental.shard_map import shard_map
    from concourse import bass2jax
    import concourse.mybir as mybir

    nc = _get_compiled()
    bass2jax.install_neuronx_cc_hook()

    partition_name = nc.partition_id_tensor.name if nc.partition_id_tensor else None
    in_names, out_names, out_avals = [], [], []
    for alloc in nc.m.functions[0].allocations:
        if not isinstance(alloc, mybir.MemoryLocationSet):
            continue
        name = alloc.memorylocations[0].name
        if alloc.kind == "ExternalInput":
            if name != partition_name:
                in_names.append(name)
        elif alloc.kind == "ExternalOutput":
            out_names.append(name)
            out_avals.append(
                jax.core.ShapedArray(tuple(alloc.tensor_shape), mybir.dt.np(alloc.dtype))
            )
    n_params = len(in_names)
    all_names = tuple(in_names) + tuple(out_names) + ((partition_name,) if partition_name else ())

    def _body(*args):
        operands = list(args)
        if partition_name is not None:
            operands.append(bass2jax.partition_id_tensor())
        outs = bass2jax._bass_exec_p.bind(
            *operands,
            out_avals=tuple(out_avals),
            in_names=all_names,
            out_names=tuple(out_names),
            lowering_input_output_aliases=(),
            sim_require_finite=True,
            sim_require_nnan=True,
            nc=nc,
        )
        return tuple(outs)

    devices = jax.devices()[:NC]
    assert len(devices) == NC, f"need {NC} devices, have {len(jax.devices())}"
    mesh = Mesh(np.asarray(devices), ("core",))
    Pc, Pr = PartitionSpec("core"), PartitionSpec()
    spec_by_name = {
        "xs": Pc, "wqT": Pc, "wkT": Pc, "wvT": Pc, "woT": Pc,
        "cosT": Pr, "sinT": Pr, "dmask": Pr,
    }
    in_specs = tuple(spec_by_name[n] for n in in_names) + (Pc,) * len(out_names)
    out_specs = (Pc,) * len(out_names)
    jitted = jax.jit(
        shard_map(_body, mesh=mesh, in_specs=in_specs, out_specs=out_specs, check_rep=False),
        donate_argnums=tuple(range(n_params, n_params + len(out_names))),
        keep_unused=True,
    )
    shard_c = NamedSharding(mesh, Pc)
    shard_r = NamedSharding(mesh, Pr)
    zeros_jit = jax.jit(
        lambda: jnp.zeros((S, HID + 4), jnp.int8), out_shardings=shard_c
    )
    _EXEC = dict(
        jitted=jitted, zeros_jit=zeros_jit, shard_c=shard_c, shard_r=shard_r,
        in_names=in_names, jax=jax, devices=devices,
    )
    return _EXEC


def _fp(a):
    a = np.asarray(a)
    if not a.flags.c_contiguous:
        a = np.ascontiguousarray(a)
    r = a.reshape(-1)
    step = max(1, r.size // 1024)
    return (
        a.dtype.str,
        a.shape,
        a.__array_interface__["data"][0],
        np.ascontiguousarray(r[::step][:1024]).tobytes(),
    )


def _rope_tables(position_ids):
    pos = np.asarray(position_ids).reshape(-1).astype(np.float32)
    inv_freq = (1.0 / (THETA ** (np.arange(0, D, 2, dtype=np.float32) / D))).astype(
        np.float32
    )
    freqs = np.outer(pos, inv_freq)
    emb = np.concatenate([freqs, freqs], axis=-1)  # [S, D]
    return np.cos(emb).astype(np.float32), np.sin(emb).astype(np.float32)


def _is_causal(mask):
    m = np.asarray(mask)[0, 0]
    if m.shape != (S, S):
        return False
    tri = np.tril(np.ones((S, S), dtype=bool))
    return bool((m[tri] == 0.0).all() and (m[~tri] < -1e30).all())


def _numpy_reference(hidden_states, attention_mask, position_ids, Wq, Wk, Wv, Wo):
    x = np.asarray(hidden_states, np.float32)
    b, s, hid = x.shape
    n_rep = H // KVH
    q = (x @ Wq.T).reshape(b, s, H, D).transpose(0, 2, 1, 3)
    k = (x @ Wk.T).reshape(b, s, KVH, D).transpose(0, 2, 1, 3)
    v = (x @ Wv.T).reshape(b, s, KVH, D).transpose(0, 2, 1, 3)
    cos_t, sin_t = _rope_tables(position_ids)
    cos = cos_t[None, None]
    sin = sin_t[None, None]

    def rot(t):
        return np.concatenate([-t[..., D // 2 :], t[..., : D // 2]], axis=-1)

    q = q * cos + rot(q) * sin
    k = k * cos + rot(k) * sin
    k = np.repeat(k, n_rep, axis=1)
    v = np.repeat(v, n_rep, axis=1)
    scores = np.einsum("bhqd,bhkd->bhqk", q, k) / np.sqrt(np.float32(D))
    scores = scores + np.asarray(attention_mask, np.float32)
    scores = scores - scores.max(axis=-1, keepdims=True)
    p = np.exp(scores)
    p = p / p.sum(axis=-1, keepdims=True)
    attn = np.einsum("bhqk,bhkd->bhqd", p, v)
    attn = attn.transpose(0, 2, 1, 3).reshape(b, s, H * D)
    return (attn @ Wo.T).astype(np.float32)


def _prep_static(position_ids, Wq, Wk, Wv, Wo):
    """Host-side weight/table prep + one-time device upload. Returns
    name->jax.Array for all static inputs."""
    ex = _get_exec()
    jax = ex["jax"]

    cos_t, sin_t = _rope_tables(position_ids)
    cosT = np.ascontiguousarray(cos_t.T).astype(_BF16)
    sinT_s = np.ascontiguousarray(sin_t.T)
    sinT_s[: D // 2] *= -1.0
    sinT_s = sinT_s.astype(_BF16)
    dm = np.zeros((D, 4 * SC), np.float32)
    ki = np.arange(D)[:, None]
    qi = np.arange(SC)[None, :]
    for j in range(4):
        dm[:, j * SC : (j + 1) * SC] = (ki <= qi - 128 * j).astype(np.float32)
    dm = dm.astype(_BF16)

    Wq32 = np.asarray(Wq, np.float32)
    Wk32 = np.asarray(Wk, np.float32)
    Wv32 = np.asarray(Wv, np.float32)
    Wo32 = np.asarray(Wo, np.float32)
    WqT = Wq32.T  # [HID, H*D]
    WkT = Wk32.T
    WvT = Wv32.T
    host = {
        "wqT": np.concatenate(
            [WqT[:, c * FC : (c + 1) * FC] for c in range(NC)], axis=0
        ).astype(_BF16),
        "wkT": np.concatenate(
            [WkT[:, c * D : (c + 1) * D] for c in range(NC)], axis=0
        ).astype(_BF16),
        "wvT": np.concatenate(
            [WvT[:, c * D : (c + 1) * D] for c in range(NC)], axis=0
        ).astype(_BF16),
        "woT": np.ascontiguousarray(Wo32.T).astype(_BF16),  # [HID, HID], rows shard
        "cosT": cosT,
        "sinT": sinT_s,
        "dmask": dm,
    }
    sharded = {"wqT", "wkT", "wvT", "woT"}
    devs = {}
    for name, arr in host.items():
        devs[name] = jax.device_put(
            arr, ex["shard_c"] if name in sharded else ex["shard_r"]
        )
    for a in devs.values():
        a.block_until_ready()
    return devs


def kernel(hidden_states, attention_mask, position_ids, Wq, Wk, Wv, Wo):
    hidden_states = np.asarray(hidden_states)

    mk = _fp(attention_mask)
    if _CAUSAL["key"] != mk:
        _CAUSAL["val"] = _is_causal(attention_mask)
        _CAUSAL["key"] = mk
    if not _CAUSAL["val"]:
        return _numpy_reference(
            hidden_states, attention_mask, position_ids, Wq, Wk, Wv, Wo
        )

    ex = _get_exec()
    jax = ex["jax"]

    skey = (_fp(position_ids), _fp(Wq), _fp(Wk), _fp(Wv), _fp(Wo))
    if _STATIC["key"] != skey:
        _STATIC["arrs"] = _prep_static(position_ids, Wq, Wk, Wv, Wo)
        _STATIC["key"] = skey

    # 12-bit per-row pack, per seq chunk so upload overlaps the packing
    x2 = np.asarray(hidden_states, np.float32).reshape(S, HID)
    devices = ex["devices"]
    HH = HID // 2
    qshards = []
    for c in range(NC):
        xc = x2[c * SCH : (c + 1) * SCH]
        am = np.abs(xc).max(axis=1)
        sc = (np.maximum(am, 1e-20) * (1.0 / 2047.0)).astype(np.float32)
        u = (np.rint(xc * (1.0 / sc)[:, None]) + 2048.0).astype(np.uint16)
        u0, u1 = u[:, :HH], u[:, HH:]
        xp = np.empty((SCH, 3 * HH + 4), np.uint8)
        xp[:, 0:HH] = u0 & 0xFF
        xp[:, HH : 2 * HH] = (u0 >> 8) | ((u1 & 0xF) << 4)
        xp[:, 2 * HH : 3 * HH] = u1 >> 4
        xp[:, 3 * HH :] = sc.reshape(-1, 1).view(np.uint8)
        qshards.append(jax.device_put(xp, devices[c]))
    xdev = jax.make_array_from_single_device_arrays(
        (S, 3 * HH + 4), ex["shard_c"], qshards
    )

    donor = _LAST["out"]
    if donor is not None:
        try:
            if donor.is_deleted():
                donor = None
        except Exception:
            donor = None
    if donor is None:
        donor = ex["zeros_jit"]()
    _LAST["out"] = None

    args = []
    for name in ex["in_names"]:
        args.append(xdev if name == "xs" else _STATIC["arrs"][name])
    (o,) = ex["jitted"](*args, donor)
    res = np.asarray(o)  # int8 [S, HID+4]
    _LAST["out"] = o
    scales = np.ascontiguousarray(res[:, HID:]).view(np.float32)
    try:
        import concurrent.futures as _cf
        global _POOL
        if _POOL is None:
            _POOL = _cf.ThreadPoolExecutor(4)
        out = np.empty((S, HID), np.float32)

        def _deq(c):
            r = slice(c * SCH, (c + 1) * SCH)
            np.multiply(res[r, :HID], scales[r], out=out[r])

        list(_POOL.map(_deq, range(NC)))
    except Exception:
        out = res[:, :HID].astype(np.float32)
        out *= scales
    return out.reshape(1, S, HID)
